# revision 32
# baseline (speedup 1.0000x reference)
"""GIN (3-layer) Trainium2 Bass kernel, 8-core SPMD.

Sharding: nodes (and their incident edges, by dst) are partitioned across the
8 cores; segment_sum is computed locally per dst shard; node features are
exchanged between layers with an AllGather; MLP weights are replicated.

Implementation sketch (per core, per layer):
  - indirect `dma_gather` pulls x[src] rows (bf16) for the core's edges from a
    padded full-node buffer in HBM (layer 0: the input x; later layers: the
    AllGather output).
  - segment-sum runs on the PE: per 128-edge chunk, a host-precomputed one-hot
    selector matrix S (bf16) is the stationary operand and the gathered rows
    are the moving operand; chunks accumulate into a PSUM tile per 128-node
    dst block -> agg[node, feat].
  - agg is transposed on the PE (128x128 blocks) and added to the resident
    fp32 h^T -> Z^T (feature-major).
  - the 2-layer MLP runs feature-major in float32r: out = W^T @ Z^T with the
    weight as the stationary operand; per-partition bias + ReLU are fused into
    the PSUM evacuation on the scalar engine.
  - h^T is transposed back to node-major, cast to bf16, DMA'd to HBM, and
    AllGathered for the next layer.
"""

import os
import sys
from contextlib import ExitStack

import numpy as np

for _p in ("/opt/trn_rl_repo", "/root/.axon_site/_ro/trn_rl_repo"):
    if os.path.isdir(_p) and _p not in sys.path:
        sys.path.append(_p)

import ml_dtypes

N_NODES = 10000
N_EDGES = 160000
D = 512
N_LAYERS = 3
CORES = 8
SHARD = N_NODES // CORES          # 1250 nodes per core
PADS = 1280                       # padded shard (multiple of 128)
PADN = CORES * PADS               # padded full node count (10240)
NB = PADS // 128                  # dst blocks per core (10)
HALF = PADS // 2                  # half-shard rows for the split AllGather

BF16 = ml_dtypes.bfloat16

# Results of the last kernel() call (BassKernelResults) for the test harness.
LAST_RESULTS = None


def _prep_host(x, edge_index, Ws, bs):
    """Per-core input maps + per-block chunk counts (uniform across cores)."""
    x = np.asarray(x, np.float32)
    src = np.asarray(edge_index[0], np.int64)
    dst = np.asarray(edge_index[1], np.int64)
    Ws = np.asarray(Ws, np.float32)
    bs = np.asarray(bs, np.float32)

    # Padded gather row index for every edge's source node.
    gidx_all = (src // SHARD) * PADS + (src % SHARD)

    owner = dst // SHARD
    li = dst % SHARD
    blk = li // 128
    slot = li - blk * 128

    # Per (core, block) unique-src counts (post-dedup) set the chunk counts.
    key = (owner * NB + blk) * PADN + gidx_all
    ucnt = np.zeros(CORES * NB, np.int64)
    kb = np.unique(key) // PADN
    np.add.at(ucnt, kb, 1)
    ucnt = ucnt.reshape(CORES, NB)
    C_list = [max(1, int(-(-ucnt[:, b].max() // 128))) for b in range(NB)]
    CMAX = max(C_list)

    # Full padded x in bf16 (pre-gather source for layer 0), host-side only.
    xg_pad = np.zeros((PADN, D), BF16)
    for o in range(CORES):
        xg_pad[o * PADS:o * PADS + SHARD] = x[o * SHARD:(o + 1) * SHARD].astype(BF16)

    Wd = np.ascontiguousarray(Ws.reshape(2 * N_LAYERS, D, D).astype(BF16))
    bT = np.ascontiguousarray(
        bs.reshape(2 * N_LAYERS, 4, 128).transpose(2, 0, 1).reshape(128, 8 * N_LAYERS))
    ident = np.eye(128, dtype=np.float32)

    order = np.lexsort((blk, owner))  # edges grouped by (owner, block)
    e_sorted = order
    bounds = np.searchsorted(owner[order] * NB + blk[order], np.arange(CORES * NB + 1))

    # ---- split-half (by src_local < HALF) chunk structure for layers 1-2.
    # Lo chunks gather from agA (AllGather of rows [0:HALF)), hi chunks from
    # agB; rows are (owner * HALF + li) resp. (owner * HALF + li - HALF).
    src_local = src % SHARD
    CL_list = [1] * NB
    CH_list = [1] * NB
    per_cb = {}
    for c in range(CORES):
        for b in range(NB):
            lo, hi = bounds[c * NB + b], bounds[c * NB + b + 1]
            e = e_sorted[lo:hi]
            mlo = src_local[e] < HALF
            eL, eH = e[mlo], e[~mlo]
            uL, iL = np.unique((src // SHARD)[eL] * HALF + src_local[eL],
                               return_inverse=True)
            uH, iH = np.unique((src // SHARD)[eH] * HALF + src_local[eH] - HALF,
                               return_inverse=True)
            per_cb[(c, b)] = (eL, uL, iL, eH, uH, iH)
            CL_list[b] = max(CL_list[b], -(-len(uL) // 128))
            CH_list[b] = max(CH_list[b], -(-len(uH) // 128))
    C12_list = [CL_list[b] + CH_list[b] for b in range(NB)]
    CMAX12 = max(C12_list)

    in_maps = []
    for c in range(CORES):
        Sd = np.zeros((NB, 128, CMAX * 128), BF16)
        S12 = np.zeros((NB, 128, CMAX12 * 128), BF16)
        idx12 = np.zeros((128, NB * CMAX12 * 8), np.int16)
        G0 = np.zeros((NB, 128, CMAX * D), BF16)
        for b in range(NB):
            C = C_list[b]
            lo, hi = bounds[c * NB + b], bounds[c * NB + b + 1]
            e = e_sorted[lo:hi]
            # Layer 0: full-block dedup; S carries multiplicity; rows come
            # pre-gathered from the host (G0), in chunk layout.
            uniq, inv = np.unique(gidx_all[e], return_inverse=True)
            n = len(uniq)
            glist = np.zeros(C * 128, np.int16)
            glist[:n] = uniq.astype(np.int16)
            np.add.at(Sd[b], (inv % 128, (inv // 128) * 128 + slot[e]), 1.0)
            G0[b, :, :C * D] = (
                xg_pad[glist.reshape(C, 128)]            # [C, 128, D]
                .transpose(1, 0, 2).reshape(128, C * D))
            # Layers 1-2: lo chunks [0, CL) then hi chunks [CL, CL+CH).
            CL, CH = CL_list[b], CH_list[b]
            eL, uL, iL, eH, uH, iH = per_cb[(c, b)]
            np.add.at(S12[b], (iL % 128, (iL // 128) * 128 + slot[eL]), 1.0)
            np.add.at(S12[b], (iH % 128, (CL + iH // 128) * 128 + slot[eH]), 1.0)
            gl = np.zeros(CL * 128, np.int16)
            gl[:len(uL)] = uL.astype(np.int16)
            gh = np.zeros(CH * 128, np.int16)
            gh[:len(uH)] = uH.astype(np.int16)
            base = b * CMAX12 * 8
            wl = gl.reshape(CL * 8, 16).T
            wh = gh.reshape(CH * 8, 16).T
            idx12[:, base:base + CL * 8] = np.tile(wl, (8, 1))
            idx12[:, base + CL * 8:base + (CL + CH) * 8] = np.tile(wh, (8, 1))
        xT_own = np.zeros((D, PADS), np.float32)
        xT_own[:, :SHARD] = x[c * SHARD:(c + 1) * SHARD].T
        in_maps.append({
            "G0": G0,
            "xT": xT_own,
            "Wd": Wd,
            "bT": bT,
            "ident": ident,
            "Sd": Sd,
            "S12": S12,
            "idx12": idx12,
        })
    return in_maps, C_list, CMAX, CL_list, CH_list, CMAX12


def build_program(C_list, CMAX, CL_list, CH_list, CMAX12):
    import concourse.bacc as bacc
    import concourse.bass as bass
    import concourse.mybir as mybir
    import concourse.tile as tile

    dt = mybir.dt
    f32, f32r, bf16, i16 = dt.float32, dt.float32r, dt.bfloat16, dt.int16
    AF = mybir.ActivationFunctionType

    nc = bacc.Bacc("TRN2", target_bir_lowering=False, debug=False,
                   enable_asserts=False, num_devices=CORES, num_swdge_queues=4)

    G0d = nc.dram_tensor("G0", [NB, 128, CMAX * D], bf16, kind="ExternalInput")
    xT = nc.dram_tensor("xT", [D, PADS], f32, kind="ExternalInput")
    Wd = nc.dram_tensor("Wd", [2 * N_LAYERS, D, D], bf16, kind="ExternalInput")
    bTd = nc.dram_tensor("bT", [128, 8 * N_LAYERS], f32, kind="ExternalInput")
    identd = nc.dram_tensor("ident", [128, 128], f32, kind="ExternalInput")
    Sd = nc.dram_tensor("Sd", [NB, 128, CMAX * 128], bf16, kind="ExternalInput")
    S12d = nc.dram_tensor("S12", [NB, 128, CMAX12 * 128], bf16, kind="ExternalInput")
    idx12d = nc.dram_tensor("idx12", [128, NB * CMAX12 * 8], i16,
                            kind="ExternalInput")
    outTd = nc.dram_tensor("outT", [D, PADS], f32, kind="ExternalOutput")

    NCHUNK = [(0, 512), (512, 128), (640, 512), (1152, 128)]  # node tiles; A-half ends at chunk 1

    with tile.TileContext(nc) as tc, ExitStack() as ctx:
        p_const = ctx.enter_context(tc.tile_pool(name="const", bufs=1))
        p_big = ctx.enter_context(tc.tile_pool(name="big", bufs=1))
        p_g = ctx.enter_context(tc.tile_pool(name="gth", bufs=10))
        p_s = ctx.enter_context(tc.tile_pool(name="sel", bufs=4))
        p_aggn = ctx.enter_context(tc.tile_pool(name="aggn", bufs=3))
        p_aggl = ctx.enter_context(tc.tile_pool(name="aggl", bufs=NB))
        p_w = ctx.enter_context(tc.tile_pool(name="wts", bufs=2))
        p_hbf = ctx.enter_context(tc.tile_pool(name="hbf", bufs=2))
        p_aggps = ctx.enter_context(tc.tile_pool(name="aggps", bufs=3, space="PSUM"))
        p_tps = ctx.enter_context(tc.tile_pool(name="tps", bufs=3, space="PSUM"))
        p_mlpps = ctx.enter_context(tc.tile_pool(name="mlpps", bufs=2, space="PSUM"))
        p_dram = ctx.enter_context(tc.tile_pool(name="dram", bufs=1, space="DRAM"))

        idxs = p_const.tile([128, NB * CMAX12 * 8], i16)
        nc.sync.dma_start(idxs[:], idx12d.ap())
        ident = p_const.tile([128, 128], f32)
        nc.sync.dma_start(ident[:], identd.ap())
        bt = p_const.tile([128, 8 * N_LAYERS], f32)
        nc.sync.dma_start(bt[:], bTd.ap())

        hT = p_big.tile([128, 4, PADS], f32)     # resident h^T (fp32)
        ZT = p_big.tile([128, 4, PADS], bf16)    # (h + agg)^T, bf16 for MLP
        Y1T = p_big.tile([128, 4, PADS], bf16)   # hidden activation^T
        for kc in range(4):
            nc.sync.dma_start(hT[:, kc, :], xT.ap()[kc * 128:(kc + 1) * 128, :])

        # Split exchange: rows [0:HALF) of each shard AllGather into agA,
        # rows [HALF:PADS) into agB — the second collective (and the hi-chunk
        # gathers) overlap the first one's consumers.
        hshA = [p_dram.tile([HALF, D], bf16, name=f"hshA{l}") for l in range(2)]
        hshB = [p_dram.tile([HALF, D], bf16, name=f"hshB{l}") for l in range(2)]
        agA = [p_dram.tile([CORES * HALF, D], bf16, addr_space="Shared",
                           name=f"agA{l}") for l in range(2)]
        agB = [p_dram.tile([CORES * HALF, D], bf16, addr_space="Shared",
                           name=f"agB{l}") for l in range(2)]

        qctr = [0]  # gather ordinal; queues rotate with Tile's DMASW lanes
        GMAX = max(max(CL_list), max(CH_list), (CMAX + 1) // 2)

        def emit_half_gather(gsrc, base, nch):
            g = p_g.tile([128, GMAX, D], bf16, tag="g", name="g")
            q = qctr[0] % 4
            qctr[0] += 1
            nc.gpsimd.dma_gather(
                out_ap=g[:, :nch, :],
                in_ap=gsrc,
                idxs_ap=idxs[:, base:base + nch * 8],
                num_idxs=nch * 128,
                num_idxs_reg=nch * 128,
                elem_size=D,
                single_packet=False,
                queue_num=q,
            )
            return g

        def emit_g0_load(b):
            """Layer 0: contiguous HWDGE loads of pre-gathered rows."""
            C = C_list[b]
            C1 = (C + 1) // 2
            gA = p_g.tile([128, GMAX, D], bf16, tag="g", name="g")
            gB = p_g.tile([128, GMAX, D], bf16, tag="g", name="g")
            nc.sync.dma_start(gA[:, :C1, :], G0d.ap()[b, :, :C1 * D])
            nc.sync.dma_start(gB[:, :C - C1, :], G0d.ap()[b, :, C1 * D:C * D])
            return (gA, gB, C1)

        for l in range(N_LAYERS):
            W0t = p_w.tile([128, 4, D], bf16, tag="w", name="W0t")
            W1t = p_w.tile([128, 4, D], bf16, tag="w", name="W1t")
            for kc in range(4):
                nc.sync.dma_start(W0t[:, kc, :], Wd.ap()[2 * l, kc * 128:(kc + 1) * 128, :])
                nc.sync.dma_start(W1t[:, kc, :], Wd.ap()[2 * l + 1, kc * 128:(kc + 1) * 128, :])

            # ---- MLP chunk / boundary-half emitters (interleaved into the
            # aggregation loop so the PE fills gather stalls and AllGather-A
            # launches before the last MLP chunk) ----
            def emit_mlp_chunk(j, i):
                nofs, nw = NCHUNK[i]
                rhs_big = ZT if j == 0 else Y1T
                Wt = W0t if j == 0 else W1t
                for mc in range(4):
                    ps2 = p_mlpps.tile([128, D], f32, tag="mlp", name="ps2")
                    for kc in range(4):
                        nc.tensor.matmul(
                            ps2[:, :nw],
                            lhsT=Wt[:, kc, mc * 128:(mc + 1) * 128],
                            rhs=rhs_big[:, kc, nofs:nofs + nw],
                            start=(kc == 0), stop=(kc == 3))
                    col = (2 * l + j) * 4 + mc
                    bias = bt[:, col:col + 1]
                    if j == 0:
                        nc.scalar.activation(Y1T[:, mc, nofs:nofs + nw],
                                             ps2[:, :nw], AF.Relu, bias=bias)
                    elif l < N_LAYERS - 1:
                        nc.scalar.activation(hT[:, mc, nofs:nofs + nw],
                                             ps2[:, :nw], AF.Relu, bias=bias)
                    else:
                        ot = p_hbf.tile([128, 512], f32, tag="ot", name="ot")
                        nc.scalar.activation(ot[:, :nw], ps2[:, :nw],
                                             AF.Identity, bias=bias)
                        nc.sync.dma_start(
                            outTd.ap()[mc * 128:(mc + 1) * 128, nofs:nofs + nw],
                            ot[:, :nw])

            def emit_boundary_half(half):
                hsh = (hshA if half == 0 else hshB)[l]
                ag = (agA if half == 0 else agB)[l]
                b0 = half * 5
                for b in range(b0, b0 + 5):
                    hb = p_hbf.tile([128, D], bf16, tag="hbf", name="hb")
                    for fc in range(4):
                        pt2 = p_tps.tile([128, 128], f32, tag="t", name="pt2")
                        nc.tensor.transpose(pt2[:],
                                            hT[:, fc, b * 128:(b + 1) * 128],
                                            ident[:])
                        nc.scalar.copy(hb[:, fc * 128:(fc + 1) * 128], pt2[:])
                    nc.sync.dma_start(
                        hsh[(b - b0) * 128:(b - b0 + 1) * 128, :], hb[:])
                nc.gpsimd.collective_compute(
                    "AllGather", mybir.AluOpType.bypass,
                    replica_groups=[list(range(CORES))],
                    ins=[hsh.opt()], outs=[ag.opt()])

            def after_evac(b_done):
                if b_done == 3:
                    emit_mlp_chunk(0, 0)
                elif b_done == 4:
                    emit_mlp_chunk(0, 1)
                    emit_mlp_chunk(1, 0)
                elif b_done == 8:
                    emit_mlp_chunk(0, 2)
                    emit_mlp_chunk(1, 1)
                    if l < N_LAYERS - 1:
                        emit_boundary_half(0)
                elif b_done == 9:
                    emit_mlp_chunk(0, 3)
                    emit_mlp_chunk(1, 2)
                    emit_mlp_chunk(1, 3)
                    if l < N_LAYERS - 1:
                        emit_boundary_half(1)

            # ---- aggregation: agg[node, feat] per 128-node dst block ----
            def emit_evac(b, ps, aNL=None):
                """PSUM (+ optional lo partial) -> aggN -> ZT^T slices."""
                aggN = p_aggn.tile([128, D], f32, name="aggN")
                if aNL is None:
                    nc.scalar.copy(aggN[:], ps[:])
                else:
                    nc.vector.tensor_add(aggN[:], ps[:], aNL[:])
                for fc in range(4):
                    pt = p_tps.tile([128, 128], f32, tag="t", name="pt")
                    nc.tensor.transpose(pt[:], aggN[:, fc * 128:(fc + 1) * 128],
                                        ident[:])
                    nc.vector.tensor_add(ZT[:, fc, b * 128:(b + 1) * 128], pt[:],
                                         hT[:, fc, b * 128:(b + 1) * 128])

            if l == 0:
                for b in range(NB):
                    C = C_list[b]
                    gA, gB, C1 = emit_g0_load(b)
                    S_b = p_s.tile([128, CMAX12, 128], bf16, tag="s", name="S_b")
                    nc.sync.dma_start(S_b[:, :C, :], Sd.ap()[b, :, :C * 128])
                    ps = p_aggps.tile([128, D], f32, name="ps")
                    for cc in range(C):
                        rhs = gA[:, cc, :] if cc < C1 else gB[:, cc - C1, :]
                        nc.tensor.matmul(ps[:], lhsT=S_b[:, cc, :], rhs=rhs,
                                         start=(cc == 0), stop=(cc == C - 1))
                    emit_evac(b, ps)
                    after_evac(b)
            else:
                # Lo pass: every block's lo-src chunks run while AllGather-B
                # is still in flight; partials are parked in SBUF.
                aggL = []
                for b in range(NB):
                    C1 = CL_list[b]
                    base = b * CMAX12 * 8
                    gA = emit_half_gather(agA[l - 1][:, :], base, C1)
                    S_b = p_s.tile([128, CMAX12, 128], bf16, tag="s", name="S_b")
                    nc.sync.dma_start(S_b[:, :C1, :], S12d.ap()[b, :, :C1 * 128])
                    ps = p_aggps.tile([128, D], f32, name="ps")
                    for cc in range(C1):
                        nc.tensor.matmul(ps[:], lhsT=S_b[:, cc, :], rhs=gA[:, cc, :],
                                         start=(cc == 0), stop=(cc == C1 - 1))
                    aNL = p_aggl.tile([128, D], f32, tag="al", name="aNL")
                    nc.scalar.copy(aNL[:], ps[:])
                    aggL.append(aNL)
                # Hi pass: hi-src chunks once AllGather-B lands, summed with
                # the lo partials during evacuation.
                for b in range(NB):
                    C1, Chi = CL_list[b], CH_list[b]
                    base = b * CMAX12 * 8 + C1 * 8
                    gB = emit_half_gather(agB[l - 1][:, :], base, Chi)
                    S_b = p_s.tile([128, CMAX12, 128], bf16, tag="s", name="S_b")
                    nc.sync.dma_start(S_b[:, :Chi, :],
                                      S12d.ap()[b, :, C1 * 128:(C1 + Chi) * 128])
                    ps = p_aggps.tile([128, D], f32, name="ps")
                    for cc in range(Chi):
                        nc.tensor.matmul(ps[:], lhsT=S_b[:, cc, :], rhs=gB[:, cc, :],
                                         start=(cc == 0), stop=(cc == Chi - 1))
                    emit_evac(b, ps, aggL[b])
                    after_evac(b)

    nc.compile()
    return nc


def kernel(**inputs):
    global LAST_RESULTS
    from concourse import bass_utils

    in_maps, C_list, CMAX, CL_list, CH_list, CMAX12 = _prep_host(
        inputs["x"], inputs["edge_index"], inputs["Ws"], inputs["bs"])
    nc = build_program(C_list, CMAX, CL_list, CH_list, CMAX12)
    res = bass_utils.run_bass_kernel_spmd(
        nc, in_maps, core_ids=list(range(CORES)),
        trace=bool(int(os.environ.get("GIN_TRACE", "0"))),
        tmpdir=os.environ.get("GIN_TMPDIR"),
    )
    LAST_RESULTS = res
    out = np.empty((N_NODES, D), np.float32)
    for c in range(CORES):
        out[c * SHARD:(c + 1) * SHARD] = res.results[c]["outT"][:, :SHARD].T
    return out



# revision 34
# speedup vs baseline: 1.0696x; 1.0696x over previous
"""GIN (3-layer) Trainium2 Bass kernel, 8-core SPMD.

Sharding: nodes (and their incident edges, by dst) are partitioned across the
8 cores; segment_sum is computed locally per dst shard; node features are
exchanged between layers with AllGathers; MLP weights are replicated.

Implementation sketch (per core, per layer):
  - layer 0: the gathered x[src] rows arrive pre-arranged from the host (G0,
    chunk layout) and stream in with plain contiguous DMA — no SWDGE gather.
  - layers 1-2: the inter-layer exchange is SPLIT into two half-shard
    AllGathers (rows [0:640) -> agA, [640:1280) -> agB). Edge chunks are
    split by src half on the host, so every block's lo-chunk `dma_gather`s
    and PE matmuls run while AllGather-B is still in flight; hi-chunk
    partials are summed with the parked lo partials at evacuation.
  - segment-sum runs on the PE: per 128-row chunk, a host-precomputed one-hot
    selector matrix S (bf16) is the stationary operand and the gathered rows
    are the moving operand, accumulating into a PSUM tile per 128-node dst
    block -> agg[node, feat]; agg is PE-transposed and added to the resident
    fp32 h^T -> Z^T (feature-major, bf16).
  - the 2-layer MLP runs feature-major with the weight as the stationary
    operand; bias + ReLU fuse into the PSUM evacuation on the scalar engine.
    MLP node-chunks are emitted interleaved into the aggregation loop (chunk
    boundaries at 512/640/1152 cols) so the PE fills gather stalls, and
    AllGather-A is issued as soon as h^T cols [0:640) exist — before the
    last MLP chunks and AllGather-B.
"""

import os
import sys
from contextlib import ExitStack

import numpy as np

for _p in ("/opt/trn_rl_repo", "/root/.axon_site/_ro/trn_rl_repo"):
    if os.path.isdir(_p) and _p not in sys.path:
        sys.path.append(_p)

import ml_dtypes

N_NODES = 10000
N_EDGES = 160000
D = 512
N_LAYERS = 3
CORES = 8
SHARD = N_NODES // CORES          # 1250 nodes per core
PADS = 1280                       # padded shard (multiple of 128)
PADN = CORES * PADS               # padded full node count (10240)
NB = PADS // 128                  # dst blocks per core (10)
HALF = PADS // 2                  # half-shard rows for the split AllGather

BF16 = ml_dtypes.bfloat16

# Results of the last kernel() call (BassKernelResults) for the test harness.
LAST_RESULTS = None


def _prep_host(x, edge_index, Ws, bs):
    """Per-core input maps + per-block chunk counts (uniform across cores)."""
    x = np.asarray(x, np.float32)
    src = np.asarray(edge_index[0], np.int64)
    dst = np.asarray(edge_index[1], np.int64)
    Ws = np.asarray(Ws, np.float32)
    bs = np.asarray(bs, np.float32)

    # Padded gather row index for every edge's source node.
    gidx_all = (src // SHARD) * PADS + (src % SHARD)

    owner = dst // SHARD
    li = dst % SHARD
    blk = li // 128
    slot = li - blk * 128

    # Per (core, block) unique-src counts (post-dedup) set the chunk counts.
    key = (owner * NB + blk) * PADN + gidx_all
    ucnt = np.zeros(CORES * NB, np.int64)
    kb = np.unique(key) // PADN
    np.add.at(ucnt, kb, 1)
    ucnt = ucnt.reshape(CORES, NB)
    C_list = [max(1, int(-(-ucnt[:, b].max() // 128))) for b in range(NB)]
    CMAX = max(C_list)

    # Full padded x in bf16 (pre-gather source for layer 0), host-side only.
    xg_pad = np.zeros((PADN, D), BF16)
    for o in range(CORES):
        xg_pad[o * PADS:o * PADS + SHARD] = x[o * SHARD:(o + 1) * SHARD].astype(BF16)

    Wd = np.ascontiguousarray(Ws.reshape(2 * N_LAYERS, D, D).astype(BF16))
    bT = np.ascontiguousarray(
        bs.reshape(2 * N_LAYERS, 4, 128).transpose(2, 0, 1).reshape(128, 8 * N_LAYERS))
    ident = np.eye(128, dtype=np.float32)

    order = np.lexsort((blk, owner))  # edges grouped by (owner, block)
    e_sorted = order
    bounds = np.searchsorted(owner[order] * NB + blk[order], np.arange(CORES * NB + 1))

    # ---- split-half (by src_local < HALF) chunk structure for layers 1-2.
    # Lo chunks gather from agA (AllGather of rows [0:HALF)), hi chunks from
    # agB; rows are (owner * HALF + li) resp. (owner * HALF + li - HALF).
    src_local = src % SHARD
    CL_list = [1] * NB
    CH_list = [1] * NB
    per_cb = {}
    for c in range(CORES):
        for b in range(NB):
            lo, hi = bounds[c * NB + b], bounds[c * NB + b + 1]
            e = e_sorted[lo:hi]
            mlo = src_local[e] < HALF
            eL, eH = e[mlo], e[~mlo]
            uL, iL = np.unique((src // SHARD)[eL] * HALF + src_local[eL],
                               return_inverse=True)
            uH, iH = np.unique((src // SHARD)[eH] * HALF + src_local[eH] - HALF,
                               return_inverse=True)
            per_cb[(c, b)] = (eL, uL, iL, eH, uH, iH)
            CL_list[b] = max(CL_list[b], -(-len(uL) // 128))
            CH_list[b] = max(CH_list[b], -(-len(uH) // 128))
    C12_list = [CL_list[b] + CH_list[b] for b in range(NB)]
    CMAX12 = max(C12_list)

    in_maps = []
    for c in range(CORES):
        Sd = np.zeros((NB, 128, CMAX * 128), BF16)
        S12 = np.zeros((NB, 128, CMAX12 * 128), BF16)
        idx12 = np.zeros((128, NB * CMAX12 * 8), np.int16)
        G0 = np.zeros((NB, 128, CMAX * D), BF16)
        for b in range(NB):
            C = C_list[b]
            lo, hi = bounds[c * NB + b], bounds[c * NB + b + 1]
            e = e_sorted[lo:hi]
            # Layer 0: full-block dedup; S carries multiplicity; rows come
            # pre-gathered from the host (G0), in chunk layout.
            uniq, inv = np.unique(gidx_all[e], return_inverse=True)
            n = len(uniq)
            glist = np.zeros(C * 128, np.int16)
            glist[:n] = uniq.astype(np.int16)
            np.add.at(Sd[b], (inv % 128, (inv // 128) * 128 + slot[e]), 1.0)
            G0[b, :, :C * D] = (
                xg_pad[glist.reshape(C, 128)]            # [C, 128, D]
                .transpose(1, 0, 2).reshape(128, C * D))
            # Layers 1-2: lo chunks [0, CL) then hi chunks [CL, CL+CH).
            CL, CH = CL_list[b], CH_list[b]
            eL, uL, iL, eH, uH, iH = per_cb[(c, b)]
            np.add.at(S12[b], (iL % 128, (iL // 128) * 128 + slot[eL]), 1.0)
            np.add.at(S12[b], (iH % 128, (CL + iH // 128) * 128 + slot[eH]), 1.0)
            gl = np.zeros(CL * 128, np.int16)
            gl[:len(uL)] = uL.astype(np.int16)
            gh = np.zeros(CH * 128, np.int16)
            gh[:len(uH)] = uH.astype(np.int16)
            base = b * CMAX12 * 8
            wl = gl.reshape(CL * 8, 16).T
            wh = gh.reshape(CH * 8, 16).T
            idx12[:, base:base + CL * 8] = np.tile(wl, (8, 1))
            idx12[:, base + CL * 8:base + (CL + CH) * 8] = np.tile(wh, (8, 1))
        xT_own = np.zeros((D, PADS), np.float32)
        xT_own[:, :SHARD] = x[c * SHARD:(c + 1) * SHARD].T
        in_maps.append({
            "G0": G0,
            "xT": xT_own,
            "Wd": Wd,
            "bT": bT,
            "ident": ident,
            "Sd": Sd,
            "S12": S12,
            "idx12": idx12,
        })
    return in_maps, C_list, CMAX, CL_list, CH_list, CMAX12


def build_program(C_list, CMAX, CL_list, CH_list, CMAX12):
    import concourse.bacc as bacc
    import concourse.bass as bass
    import concourse.mybir as mybir
    import concourse.tile as tile

    dt = mybir.dt
    f32, f32r, bf16, i16 = dt.float32, dt.float32r, dt.bfloat16, dt.int16
    AF = mybir.ActivationFunctionType

    nc = bacc.Bacc("TRN2", target_bir_lowering=False, debug=False,
                   enable_asserts=False, num_devices=CORES, num_swdge_queues=4)

    G0d = nc.dram_tensor("G0", [NB, 128, CMAX * D], bf16, kind="ExternalInput")
    xT = nc.dram_tensor("xT", [D, PADS], f32, kind="ExternalInput")
    Wd = nc.dram_tensor("Wd", [2 * N_LAYERS, D, D], bf16, kind="ExternalInput")
    bTd = nc.dram_tensor("bT", [128, 8 * N_LAYERS], f32, kind="ExternalInput")
    identd = nc.dram_tensor("ident", [128, 128], f32, kind="ExternalInput")
    Sd = nc.dram_tensor("Sd", [NB, 128, CMAX * 128], bf16, kind="ExternalInput")
    S12d = nc.dram_tensor("S12", [NB, 128, CMAX12 * 128], bf16, kind="ExternalInput")
    idx12d = nc.dram_tensor("idx12", [128, NB * CMAX12 * 8], i16,
                            kind="ExternalInput")
    outTd = nc.dram_tensor("outT", [D, PADS], f32, kind="ExternalOutput")

    NCHUNK = [(0, 512), (512, 128), (640, 512), (1152, 128)]  # node tiles; A-half ends at chunk 1

    with tile.TileContext(nc) as tc, ExitStack() as ctx:
        p_const = ctx.enter_context(tc.tile_pool(name="const", bufs=1))
        p_big = ctx.enter_context(tc.tile_pool(name="big", bufs=1))
        p_g = ctx.enter_context(tc.tile_pool(name="gth", bufs=10))
        p_s = ctx.enter_context(tc.tile_pool(name="sel", bufs=4))
        p_aggn = ctx.enter_context(tc.tile_pool(name="aggn", bufs=3))
        p_aggl = ctx.enter_context(tc.tile_pool(name="aggl", bufs=NB))
        p_w = ctx.enter_context(tc.tile_pool(name="wts", bufs=2))
        p_hbf = ctx.enter_context(tc.tile_pool(name="hbf", bufs=2))
        p_aggps = ctx.enter_context(tc.tile_pool(name="aggps", bufs=3, space="PSUM"))
        p_tps = ctx.enter_context(tc.tile_pool(name="tps", bufs=3, space="PSUM"))
        p_mlpps = ctx.enter_context(tc.tile_pool(name="mlpps", bufs=2, space="PSUM"))
        p_dram = ctx.enter_context(tc.tile_pool(name="dram", bufs=1, space="DRAM"))

        idxs = p_const.tile([128, NB * CMAX12 * 8], i16)
        nc.sync.dma_start(idxs[:], idx12d.ap())
        ident = p_const.tile([128, 128], f32)
        nc.sync.dma_start(ident[:], identd.ap())
        bt = p_const.tile([128, 8 * N_LAYERS], f32)
        nc.sync.dma_start(bt[:], bTd.ap())

        hT = p_big.tile([128, 4, PADS], f32)     # resident h^T (fp32)
        ZT = p_big.tile([128, 4, PADS], bf16)    # (h + agg)^T, bf16 for MLP
        Y1T = p_big.tile([128, 4, PADS], bf16)   # hidden activation^T
        for kc in range(4):
            nc.sync.dma_start(hT[:, kc, :], xT.ap()[kc * 128:(kc + 1) * 128, :])

        wa_in = p_dram.tile([128, D], bf16, name="wa_in")
        wa_out = p_dram.tile([128 * CORES, D], bf16, addr_space="Shared", name="wa_out")
        nc.sync.dma_start(wa_in[:, :], xT.ap()[0:128, 0:D].bitcast(bf16)[:, 0:D])
        nc.gpsimd.collective_compute(
            "AllGather", mybir.AluOpType.bypass,
            replica_groups=[list(range(CORES))],
            ins=[wa_in.opt()], outs=[wa_out.opt()])

        # Split exchange: rows [0:HALF) of each shard AllGather into agA,
        # rows [HALF:PADS) into agB — the second collective (and the hi-chunk
        # gathers) overlap the first one's consumers.
        hshA = [p_dram.tile([HALF, D], bf16, name=f"hshA{l}") for l in range(2)]
        hshB = [p_dram.tile([HALF, D], bf16, name=f"hshB{l}") for l in range(2)]
        agA = [p_dram.tile([CORES * HALF, D], bf16, addr_space="Shared",
                           name=f"agA{l}") for l in range(2)]
        agB = [p_dram.tile([CORES * HALF, D], bf16, addr_space="Shared",
                           name=f"agB{l}") for l in range(2)]

        qctr = [0]  # gather ordinal; queues rotate with Tile's DMASW lanes
        GMAX = max(max(CL_list), max(CH_list), (CMAX + 1) // 2)

        def emit_half_gather(gsrc, base, nch):
            g = p_g.tile([128, GMAX, D], bf16, tag="g", name="g")
            q = qctr[0] % 4
            qctr[0] += 1
            nc.gpsimd.dma_gather(
                out_ap=g[:, :nch, :],
                in_ap=gsrc,
                idxs_ap=idxs[:, base:base + nch * 8],
                num_idxs=nch * 128,
                num_idxs_reg=nch * 128,
                elem_size=D,
                single_packet=False,
                queue_num=q,
            )
            return g

        def emit_g0_load(b):
            """Layer 0: contiguous HWDGE loads of pre-gathered rows."""
            C = C_list[b]
            C1 = (C + 1) // 2
            gA = p_g.tile([128, GMAX, D], bf16, tag="g", name="g")
            gB = p_g.tile([128, GMAX, D], bf16, tag="g", name="g")
            nc.sync.dma_start(gA[:, :C1, :], G0d.ap()[b, :, :C1 * D])
            nc.sync.dma_start(gB[:, :C - C1, :], G0d.ap()[b, :, C1 * D:C * D])
            return (gA, gB, C1)

        for l in range(N_LAYERS):
            W0t = p_w.tile([128, 4, D], bf16, tag="w", name="W0t")
            W1t = p_w.tile([128, 4, D], bf16, tag="w", name="W1t")
            for kc in range(4):
                nc.sync.dma_start(W0t[:, kc, :], Wd.ap()[2 * l, kc * 128:(kc + 1) * 128, :])
                nc.sync.dma_start(W1t[:, kc, :], Wd.ap()[2 * l + 1, kc * 128:(kc + 1) * 128, :])

            # ---- MLP chunk / boundary-half emitters (interleaved into the
            # aggregation loop so the PE fills gather stalls and AllGather-A
            # launches before the last MLP chunk) ----
            def emit_mlp_chunk(j, i):
                nofs, nw = NCHUNK[i]
                rhs_big = ZT if j == 0 else Y1T
                Wt = W0t if j == 0 else W1t
                for mc in range(4):
                    ps2 = p_mlpps.tile([128, D], f32, tag="mlp", name="ps2")
                    for kc in range(4):
                        nc.tensor.matmul(
                            ps2[:, :nw],
                            lhsT=Wt[:, kc, mc * 128:(mc + 1) * 128],
                            rhs=rhs_big[:, kc, nofs:nofs + nw],
                            start=(kc == 0), stop=(kc == 3))
                    col = (2 * l + j) * 4 + mc
                    bias = bt[:, col:col + 1]
                    if j == 0:
                        nc.scalar.activation(Y1T[:, mc, nofs:nofs + nw],
                                             ps2[:, :nw], AF.Relu, bias=bias)
                    elif l < N_LAYERS - 1:
                        nc.scalar.activation(hT[:, mc, nofs:nofs + nw],
                                             ps2[:, :nw], AF.Relu, bias=bias)
                    else:
                        ot = p_hbf.tile([128, 512], f32, tag="ot", name="ot")
                        nc.scalar.activation(ot[:, :nw], ps2[:, :nw],
                                             AF.Identity, bias=bias)
                        nc.sync.dma_start(
                            outTd.ap()[mc * 128:(mc + 1) * 128, nofs:nofs + nw],
                            ot[:, :nw])

            def emit_boundary_half(half):
                hsh = (hshA if half == 0 else hshB)[l]
                ag = (agA if half == 0 else agB)[l]
                b0 = half * 5
                for b in range(b0, b0 + 5):
                    hb = p_hbf.tile([128, D], bf16, tag="hbf", name="hb")
                    for fc in range(4):
                        pt2 = p_tps.tile([128, 128], f32, tag="t", name="pt2")
                        nc.tensor.transpose(pt2[:],
                                            hT[:, fc, b * 128:(b + 1) * 128],
                                            ident[:])
                        nc.scalar.copy(hb[:, fc * 128:(fc + 1) * 128], pt2[:])
                    nc.sync.dma_start(
                        hsh[(b - b0) * 128:(b - b0 + 1) * 128, :], hb[:])
                nc.gpsimd.collective_compute(
                    "AllGather", mybir.AluOpType.bypass,
                    replica_groups=[list(range(CORES))],
                    ins=[hsh.opt()], outs=[ag.opt()])

            def after_evac(b_done):
                if b_done == 3:
                    emit_mlp_chunk(0, 0)
                elif b_done == 4:
                    emit_mlp_chunk(0, 1)
                    emit_mlp_chunk(1, 0)
                elif b_done == 8:
                    emit_mlp_chunk(0, 2)
                    emit_mlp_chunk(1, 1)
                    if l < N_LAYERS - 1:
                        emit_boundary_half(0)
                elif b_done == 9:
                    emit_mlp_chunk(0, 3)
                    emit_mlp_chunk(1, 2)
                    emit_mlp_chunk(1, 3)
                    if l < N_LAYERS - 1:
                        emit_boundary_half(1)

            # ---- aggregation: agg[node, feat] per 128-node dst block ----
            def emit_evac(b, ps, aNL=None):
                """PSUM (+ optional lo partial) -> aggN -> ZT^T slices."""
                aggN = p_aggn.tile([128, D], f32, name="aggN")
                if aNL is None:
                    nc.scalar.copy(aggN[:], ps[:])
                else:
                    nc.vector.tensor_add(aggN[:], ps[:], aNL[:])
                for fc in range(4):
                    pt = p_tps.tile([128, 128], f32, tag="t", name="pt")
                    nc.tensor.transpose(pt[:], aggN[:, fc * 128:(fc + 1) * 128],
                                        ident[:])
                    nc.vector.tensor_add(ZT[:, fc, b * 128:(b + 1) * 128], pt[:],
                                         hT[:, fc, b * 128:(b + 1) * 128])

            if l == 0:
                for b in range(NB):
                    C = C_list[b]
                    gA, gB, C1 = emit_g0_load(b)
                    S_b = p_s.tile([128, CMAX12, 128], bf16, tag="s", name="S_b")
                    nc.sync.dma_start(S_b[:, :C, :], Sd.ap()[b, :, :C * 128])
                    ps = p_aggps.tile([128, D], f32, name="ps")
                    for cc in range(C):
                        rhs = gA[:, cc, :] if cc < C1 else gB[:, cc - C1, :]
                        nc.tensor.matmul(ps[:], lhsT=S_b[:, cc, :], rhs=rhs,
                                         start=(cc == 0), stop=(cc == C - 1))
                    emit_evac(b, ps)
                    after_evac(b)
            else:
                # Lo pass: every block's lo-src chunks run while AllGather-B
                # is still in flight; partials are parked in SBUF.
                aggL = []
                for b in range(NB):
                    C1 = CL_list[b]
                    base = b * CMAX12 * 8
                    gA = emit_half_gather(agA[l - 1][:, :], base, C1)
                    S_b = p_s.tile([128, CMAX12, 128], bf16, tag="s", name="S_b")
                    nc.sync.dma_start(S_b[:, :C1, :], S12d.ap()[b, :, :C1 * 128])
                    ps = p_aggps.tile([128, D], f32, name="ps")
                    for cc in range(C1):
                        nc.tensor.matmul(ps[:], lhsT=S_b[:, cc, :], rhs=gA[:, cc, :],
                                         start=(cc == 0), stop=(cc == C1 - 1))
                    aNL = p_aggl.tile([128, D], f32, tag="al", name="aNL")
                    nc.scalar.copy(aNL[:], ps[:])
                    aggL.append(aNL)
                # Hi pass: hi-src chunks once AllGather-B lands, summed with
                # the lo partials during evacuation.
                for b in range(NB):
                    C1, Chi = CL_list[b], CH_list[b]
                    base = b * CMAX12 * 8 + C1 * 8
                    gB = emit_half_gather(agB[l - 1][:, :], base, Chi)
                    S_b = p_s.tile([128, CMAX12, 128], bf16, tag="s", name="S_b")
                    nc.sync.dma_start(S_b[:, :Chi, :],
                                      S12d.ap()[b, :, C1 * 128:(C1 + Chi) * 128])
                    ps = p_aggps.tile([128, D], f32, name="ps")
                    for cc in range(Chi):
                        nc.tensor.matmul(ps[:], lhsT=S_b[:, cc, :], rhs=gB[:, cc, :],
                                         start=(cc == 0), stop=(cc == Chi - 1))
                    emit_evac(b, ps, aggL[b])
                    after_evac(b)

    nc.compile()
    return nc


def kernel(**inputs):
    global LAST_RESULTS
    from concourse import bass_utils

    in_maps, C_list, CMAX, CL_list, CH_list, CMAX12 = _prep_host(
        inputs["x"], inputs["edge_index"], inputs["Ws"], inputs["bs"])
    nc = build_program(C_list, CMAX, CL_list, CH_list, CMAX12)
    res = bass_utils.run_bass_kernel_spmd(
        nc, in_maps, core_ids=list(range(CORES)),
        trace=bool(int(os.environ.get("GIN_TRACE", "0"))),
        tmpdir=os.environ.get("GIN_TMPDIR"),
    )
    LAST_RESULTS = res
    out = np.empty((N_NODES, D), np.float32)
    for c in range(CORES):
        out[c * SHARD:(c + 1) * SHARD] = res.results[c]["outT"][:, :SHARD].T
    return out

